# revision 1
# baseline (speedup 1.0000x reference)
"""TRN2 Bass kernel for nn_Blur: upfirdn2d(pad=(2,1)) with a separable 4x4
binomial FIR, x shape (8, 256, 256, 256) f32, depthwise per (n, c) plane.

Strategy
--------
Batch-parallel across the 8 NeuronCores (core i gets x[i]).

The FIR is separable: out = T_H^T @ X @ T_W per channel plane, where
T_H/T_W are 256x256 banded Toeplitz matrices (band k1[0..3] on diagonals
-1..+2, zero boundary = the reference's zero padding).

Both passes run on the TensorEngine with the *data* as the stationary
operand (lhsT) and the Toeplitz as the moving operand (rhs):

  pass1:  Y^T = X^T @ T_H      (lhsT = X tile   [h_in=128, w=128],
                                rhs  = T_H blk  [h_in=128, h'=256])
  pass2:  Z   = Y  @ T_W       (lhsT = Y^T tile [w_in=128, h'=128],
                                rhs  = T_W blk  [w_in=128, w'=256])

so no transposes are needed: pass1 naturally yields Y^T, pass2 naturally
yields Z in output layout.

Precision: the rel-err budget is 2e-2; plain bf16 data (~1e-3 end to end)
is comfortably inside it, so x is cast to bf16 on the host and every
device tensor except PSUM is bf16 — half the HBM traffic and half the PE
work of an fp32-exact split scheme.

DMA efficiency: both input and output DRAM tensors are laid out so each
SBUF partition's slice is one contiguous 16 KiB DRAM run per group of
CG=16 channels (input pre-swizzled on the host, output gathered on the
host), keeping descriptor counts minimal.  The banded structure makes
each Toeplitz 128-block touch a single contiguous column interval
([0,130) / [127,256)), so each accumulation needs just 2 matmuls: one
full-width N=256 (start=True zero-fills the tail) + one N=129.
"""
import numpy as np
import ml_dtypes

import concourse.bacc as bacc
import concourse.mybir as mybir
from concourse.tile import TileContext
from concourse.bass_utils import run_bass_kernel_spmd

N, C, H, W = 8, 256, 256, 256
P = 128          # partition size
NCORES = 8
# band: T[i, i+d] = k1[d+1], d in {-1, 0, 1, 2}
BAND_LO, BAND_HI = -1, 2
# nonzero column ranges of the two 128-row Toeplitz blocks (natural order)
BLK_COLS = [(0, P + BAND_HI), (P + BAND_LO, 2 * P)]   # [0,130), [127,256)

CG = 16          # channels per DMA group

_CACHE = {}


def _factor_kernel(k2: np.ndarray):
    """Rank-1 factorization k2 = kh (x) kw (float64)."""
    k2 = np.asarray(k2, dtype=np.float64)
    u, s, vt = np.linalg.svd(k2)
    kh = u[:, 0] * np.sqrt(s[0])
    kw = vt[0] * np.sqrt(s[0])
    if kh.sum() < 0:
        kh, kw = -kh, -kw
    return kh, kw


def _toeplitz(n: int, k1: np.ndarray) -> np.ndarray:
    """T[i, j] = k1[j - i + 1] for 0 <= j-i+1 < 4, zero elsewhere."""
    t = np.zeros((n, n), dtype=np.float64)
    for d in range(BAND_LO, BAND_HI + 1):
        i = np.arange(max(0, -d), min(n, n - d))
        t[i, i + d] = k1[d + 1]
    return t


def _build(n_ch: int, cg: int = CG, reps: int = 1, *,
           dma_split: bool = True, skip_compute: bool = False,
           skip_dma: bool = False, bufs: tuple = (4, 5),
           only: str | None = None, skew: int = 2,
           p2alt: bool = False, swap_q: bool = False, alt_q: bool = False,
           hstore: int = 1, psum_bufs: int = 4):
    """Build + compile the per-core Bass program (SPMD, one core's slice).

    reps > 1 repeats the whole channel loop (idempotent) — a timing aid
    that amortizes dispatch overhead out of wall-clock measurements.
    dma_split: issue stores on the Activation HWDGE queue (loads stay on
    SP) so the two big streams ride different hardware queues.
    skip_compute / skip_dma: ablation variants for bottleneck attribution.
    """
    nc = bacc.Bacc("TRN2", target_bir_lowering=False)

    bf16 = mybir.dt.bfloat16

    assert n_ch % cg == 0
    ng = n_ch // cg
    # [group][partition][c][hb][w] pre-swizzled bf16 input
    xin = nc.declare_dram_parameter("xin", [ng, P, cg * 2 * W], bf16,
                                    isOutput=False)
    th = nc.declare_dram_parameter("th", [2, P, H], bf16, isOutput=False)
    tw = nc.declare_dram_parameter("tw", [2, P, W], bf16, isOutput=False)
    # [group][partition][c][s][w] partition-major output: h = s*128 + p
    out = nc.declare_dram_parameter("out", [ng, P, cg * 2 * W], bf16,
                                    isOutput=True)

    with TileContext(nc) as tc:
        with (tc.tile_pool(name="const", bufs=1) as cpool,
              tc.tile_pool(name="xin_p", bufs=bufs[0]) as xpool,
              tc.tile_pool(name="mid", bufs=6) as mpool,
              tc.tile_pool(name="zout", bufs=bufs[1]) as zpool,
              tc.tile_pool(name="psy", bufs=psum_bufs, space="PSUM") as pypool,
              tc.tile_pool(name="psz", bufs=psum_bufs, space="PSUM") as pzpool):

            tth = [cpool.tile([P, H], bf16, name=f"tth{b}", tag=f"tth{b}")
                   for b in range(2)]
            ttw = [cpool.tile([P, W], bf16, name=f"ttw{b}", tag=f"ttw{b}")
                   for b in range(2)]
            for b in range(2):
                nc.sync.dma_start(out=tth[b][:, :], in_=th[b])
                nc.sync.dma_start(out=ttw[b][:, :], in_=tw[b])

            f32 = mybir.dt.float32

            def emit_group(g):
                # one contiguous load: [128 x 16 KiB]
                tx = xpool.tile([P, cg * 2 * W], bf16, name="tx", tag="tx")
                if not skip_dma and only != "store":
                    if alt_q:
                        leng = nc.sync if g % 2 == 0 else nc.scalar
                    else:
                        leng = nc.scalar if swap_q else nc.sync
                    leng.dma_start(out=tx[:, :], in_=xin[g])

                tz = zpool.tile([P, cg * 2 * W], bf16, name="tz", tag="tz")
                if skip_compute and only == "store":
                    nc.vector.memset(tz[:, :], 0.0)

                def pass1(ci):
                    # ---- pass1: Y^T[wb] = sum_hb X[hb,:,wb]^T @ TH[hb]
                    # py[:, wb*256 + h'] = Y[h', wb*128 + p]
                    py = pypool.tile([P, 2 * H], f32, name="py", tag="py")
                    for wb in range(2):
                        for hb in range(2):
                            lo, hi = (0, H) if hb == 0 else BLK_COLS[1]
                            off = (ci * 2 + hb) * W + wb * P
                            nc.tensor.matmul(
                                py[:, wb * H + lo:wb * H + hi],
                                tx[:, off:off + P],
                                tth[hb][:, lo:hi],
                                start=(hb == 0), stop=(hb == 1))
                    ty = mpool.tile([P, 2 * H], bf16, name="ty", tag="ty")
                    nc.vector.tensor_copy(ty[:, :], py[:, :])
                    return ty

                def pass2(ci, ty):
                    # ---- pass2: Z[s*128+p, w'] = sum_wb Y^T[wb,:,s]^T @ TW[wb]
                    # pz[:, s*256 + w'] = Z[s*128 + p, w']
                    pz = pzpool.tile([P, 2 * W], f32, name="pz", tag="pz")
                    for s in range(2):
                        for wb in range(2):
                            lo, hi = (0, W) if wb == 0 else BLK_COLS[1]
                            nc.tensor.matmul(
                                pz[:, s * W + lo:s * W + hi],
                                ty[:, wb * H + s * P:wb * H + s * P + P],
                                ttw[wb][:, lo:hi],
                                start=(wb == 0), stop=(wb == 1))
                    dst = tz[:, ci * 2 * W:(ci + 1) * 2 * W]
                    if p2alt and ci % 2:
                        nc.vector.tensor_copy(dst, pz[:, :])
                    else:
                        nc.scalar.copy(dst, pz[:, :])

                # software-pipeline: pass1 of channel ci+skew runs ahead of
                # pass2 of channel ci so the PE never waits on the DVE
                # PSUM->SBUF copy between passes (PE queue is in-order).
                pend = []
                for ci in range(cg) if not skip_compute else []:
                    pend.append((ci, pass1(ci)))
                    if len(pend) > skew:
                        pass2(*pend.pop(0))
                for item in pend:
                    pass2(*item)

                # ---- store: one contiguous [128 x 16 KiB] run
                if not skip_dma and only != "load":
                    src = tx if (skip_compute and only != "store") else tz
                    if alt_q:
                        eng = nc.scalar if g % 2 == 0 else nc.sync
                    elif not dma_split:
                        eng = nc.sync
                    else:
                        eng = nc.sync if swap_q else nc.scalar
                    # hstore > 1: slice the group store so the first slice
                    # fires as soon as its channels' copies land, smoothing
                    # the (bandwidth-critical) HBM write stream
                    step = cg * 2 * W // hstore
                    for si in range(hstore):
                        eng.dma_start(out=out[g][:, si * step:(si + 1) * step],
                                      in_=src[:, si * step:(si + 1) * step])

            if reps > 1:
                # hardware loop: repeat the (idempotent) channel loop
                # in-NEFF for wall-clock timing without code growth
                with tc.For_i(0, reps, 1):
                    for g in range(ng):
                        emit_group(g)
            else:
                for g in range(ng):
                    emit_group(g)
    nc.compile()
    return nc


def _get_nc(n_ch: int):
    key = (n_ch, CG)
    if key not in _CACHE:
        _CACHE[key] = _build(n_ch)
    return _CACHE[key]


def _prep_inputs(x: np.ndarray, k2: np.ndarray, n_ch: int, cg: int = CG):
    ng = n_ch // cg
    kh, kw = _factor_kernel(k2)
    th = _toeplitz(H, kh).astype(ml_dtypes.bfloat16).reshape(2, P, H)
    tw = _toeplitz(W, kw).astype(ml_dtypes.bfloat16).reshape(2, P, W)
    th = np.ascontiguousarray(th)
    tw = np.ascontiguousarray(tw)

    xb = np.asarray(x).astype(ml_dtypes.bfloat16)
    # [n, (g c), (hb p), w] -> [n, g, p, c, hb, w]
    xb = xb.reshape(N, ng, cg, 2, P, W).transpose(0, 1, 4, 2, 3, 5)
    xin = np.ascontiguousarray(xb).reshape(N, ng, P, cg * 2 * W)

    in_maps = []
    for i in range(NCORES):
        in_maps.append({"xin": xin[i], "th": th, "tw": tw})
    return in_maps


def _unpack_out(raw: np.ndarray, n_ch: int, cg: int = CG) -> np.ndarray:
    """[ng, P, cg*2*W] bf16 -> [n_ch, H, W] f32; h = s*128 + p."""
    ng = n_ch // cg
    r = raw.reshape(ng, P, cg, 2, W).transpose(0, 2, 3, 1, 4)
    return np.ascontiguousarray(r).reshape(n_ch, H, W).astype(np.float32)


def _run(x: np.ndarray, k2: np.ndarray, trace: bool = False):
    n_ch = C
    nc = _get_nc(n_ch)
    in_maps = _prep_inputs(x, k2, n_ch)
    r = run_bass_kernel_spmd(nc, in_maps, core_ids=list(range(NCORES)),
                             trace=trace)
    outs = [_unpack_out(r.results[i]["out"], n_ch) for i in range(NCORES)]
    return np.stack(outs, axis=0), r


def kernel(x: np.ndarray, kernel: np.ndarray) -> np.ndarray:
    out, _ = _run(x, kernel, trace=False)
    return out



# revision 9
# speedup vs baseline: 1.3888x; 1.3888x over previous
"""TRN2 Bass kernel for nn_Blur: upfirdn2d(pad=(2,1)) with a separable 4x4
binomial FIR, x shape (8, 256, 256, 256) f32, depthwise per (n, c) plane.

Strategy (v2)
-------------
Batch-parallel across the 8 NeuronCores (core i gets x[i]).

Separable FIR as banded-Toeplitz matmuls on the TensorEngine, data
stationary in both passes (transpose-free):

  pass1:  Y^T = X^T @ T_H      pass2:  Z = Y @ T_W

v2 changes over the 223 us baseline:
- Band-pruned matmuls: each accumulation covers only the nonzero column
  ranges [0,130) (start=True) and [127,256) (start=False; has_written
  bits make the disjoint part overwrite and the 3-col overlap
  accumulate).  518 streamed cols per pass per channel instead of 770.
- int8 output: the per-channel quantization scale so_c = ALPHA*max|x_c|
  is folded into the host-side input scaling (x/so_c shipped as bf16),
  so the device just converts PSUM f32 -> int8 (hardware convert is
  round-to-nearest-even with saturation, probed).  Host dequantizes.
  Cuts store traffic 2x: 48 MiB/core total @ ~358 GB/s -> ~140 us.
- PSUM->SBUF evacuations batched to FD=1024 (two channels per
  instruction, PSUM tiles span 2 banks) and alternated between the DVE
  and ACT engines so neither is the bottleneck.
- cg=32 channels per DMA group (4 MiB loads / 2 MiB stores); loads on
  the SP HWDGE queue, stores on the Pool SWDGE queue so no compute
  engine's sequencer pays DMA-descriptor generation time.
"""
import numpy as np
import ml_dtypes

import concourse.bacc as bacc
import concourse.mybir as mybir
from concourse.tile import TileContext
from concourse.bass_utils import run_bass_kernel_spmd

N, C, H, W = 8, 256, 256, 256
P = 128          # partition size
NCORES = 8
# band: T[i, i+d] = k1[d+1], d in {-1, 0, 1, 2}
BAND_LO, BAND_HI = -1, 2
# nonzero column ranges of the two 128-row Toeplitz blocks
BLK_COLS = [(0, P + BAND_HI), (P + BAND_LO, 2 * P)]   # [0,130), [127,256)

CG = 32          # channels per DMA group
ALPHA = 0.010    # int8 output scale: so_c = ALPHA * max|x_c|

_CACHE = {}


def _factor_kernel(k2: np.ndarray):
    """Rank-1 factorization k2 = kh (x) kw (float64)."""
    k2 = np.asarray(k2, dtype=np.float64)
    u, s, vt = np.linalg.svd(k2)
    kh = u[:, 0] * np.sqrt(s[0])
    kw = vt[0] * np.sqrt(s[0])
    if kh.sum() < 0:
        kh, kw = -kh, -kw
    return kh, kw


def _toeplitz(n: int, k1: np.ndarray) -> np.ndarray:
    """T[i, j] = k1[j - i + 1] for 0 <= j-i+1 < 4, zero elsewhere."""
    t = np.zeros((n, n), dtype=np.float64)
    for d in range(BAND_LO, BAND_HI + 1):
        i = np.arange(max(0, -d), min(n, n - d))
        t[i, i + d] = k1[d + 1]
    return t


def _build(n_ch: int, cg: int = CG, reps: int = 1, *,
           bufs: tuple = (3, 4, 3), store_eng: str = "gpsimd",
           load_eng: str = "sync", hload: int = 4, hstore: int = 2,
           act_extra: tuple = (7,),
           skip_compute: bool = False, skip_dma: bool = False):
    """Build + compile the per-core Bass program (SPMD, one core's slice).

    reps > 1 repeats the whole channel loop (idempotent) in a hardware
    loop - a timing aid that amortizes dispatch overhead.
    """
    nc = bacc.Bacc("TRN2", target_bir_lowering=False)

    bf16 = mybir.dt.bfloat16
    f32 = mybir.dt.float32
    i8 = mybir.dt.int8

    assert n_ch % cg == 0
    ng = n_ch // cg
    # [group][partition][c][hb][w] pre-swizzled, pre-scaled bf16 input
    xin = nc.declare_dram_parameter("xin", [ng, P, cg * 2 * W], bf16,
                                    isOutput=False)
    th = nc.declare_dram_parameter("th", [2, P, H], bf16, isOutput=False)
    tw = nc.declare_dram_parameter("tw", [2, P, W], bf16, isOutput=False)
    # [group][partition][c][s][w] int8 output: h = s*128 + p, value
    # out[c,h,w]/so_c rounded (RNE, saturating)
    out = nc.declare_dram_parameter("out", [ng, P, cg * 2 * W], i8,
                                    isOutput=True)

    (lo0, hi0), (lo1, hi1) = BLK_COLS

    with TileContext(nc) as tc:
        with (tc.tile_pool(name="const", bufs=1) as cpool,
              tc.tile_pool(name="xin_p", bufs=bufs[0]) as xpool,
              tc.tile_pool(name="mid", bufs=bufs[1]) as mpool,
              tc.tile_pool(name="zout", bufs=bufs[2]) as zpool,
              tc.tile_pool(name="psy", bufs=2, space="PSUM") as pypool,
              tc.tile_pool(name="psz", bufs=2, space="PSUM") as pzpool):

            tth = [cpool.tile([P, H], bf16, name=f"tth{b}", tag=f"tth{b}")
                   for b in range(2)]
            ttw = [cpool.tile([P, W], bf16, name=f"ttw{b}", tag=f"ttw{b}")
                   for b in range(2)]
            for b in range(2):
                nc.sync.dma_start(out=tth[b][:, :], in_=th[b])
                nc.sync.dma_start(out=ttw[b][:, :], in_=tw[b])

            npairs = cg // 2

            def emit_loads(g):
                # load [128 x 32 KiB], sliced so compute starts after the
                # first slice lands instead of after the full 4 MiB
                tx = xpool.tile([P, cg * 2 * W], bf16, name="tx", tag="tx")
                if not skip_dma:
                    leng = getattr(nc, load_eng)
                    lstep = cg * 2 * W // hload
                    for si in range(hload):
                        leng.dma_start(
                            out=tx[:, si * lstep:(si + 1) * lstep],
                            in_=xin[g][:, si * lstep:(si + 1) * lstep])
                tz = zpool.tile([P, cg * 2 * W], i8, name="tz", tag="tz")
                if skip_compute:
                    nc.vector.memset(tz[:, :], 0)
                return tx, tz

            def pass1(tx, j):
                # two channels (2j, 2j+1) -> py[:, q*512 + wb*256 + h']
                py = pypool.tile([P, 1024], f32, name="py", tag="py")
                for q in range(2):
                    ci = 2 * j + q
                    for wb in range(2):
                        dst = q * 512 + wb * H
                        off = ci * 2 * W + wb * P
                        nc.tensor.matmul(
                            py[:, dst + lo0:dst + hi0],
                            tx[:, off:off + P],
                            tth[0][:, lo0:hi0],
                            start=True, stop=False)
                        nc.tensor.matmul(
                            py[:, dst + lo1:dst + hi1],
                            tx[:, off + W:off + W + P],
                            tth[1][:, lo1:hi1],
                            start=False, stop=True)
                return py

            def evac1(j, py):
                ty = mpool.tile([P, 1024], bf16, name="ty", tag="ty")
                if j % 2 == 0:
                    nc.vector.tensor_copy(ty[:, :], py[:, :])
                else:
                    nc.scalar.copy(ty[:, :], py[:, :])
                return ty

            def pass2(j, ty):
                # pz[:, q*512 + s*256 + w'] ; partitions = h' in s
                pz = pzpool.tile([P, 1024], f32, name="pz", tag="pz")
                for q in range(2):
                    for s in range(2):
                        dst = q * 512 + s * W
                        nc.tensor.matmul(
                            pz[:, dst + lo0:dst + hi0],
                            ty[:, q * 512 + 0 * H + s * P:
                               q * 512 + 0 * H + s * P + P],
                            ttw[0][:, lo0:hi0],
                            start=True, stop=False)
                        nc.tensor.matmul(
                            pz[:, dst + lo1:dst + hi1],
                            ty[:, q * 512 + 1 * H + s * P:
                               q * 512 + 1 * H + s * P + P],
                            ttw[1][:, lo1:hi1],
                            start=False, stop=True)
                return pz

            def evac2(j, pz, tz):
                # ACT's copies are cheaper; act_extra shifts extra pairs
                # to ACT so both engines finish together (~15/17 split)
                dst = tz[:, (2 * j) * 512:(2 * j + 2) * 512]
                if j % 2 == 0 or j in act_extra:
                    nc.scalar.copy(dst, pz[:, :])
                else:
                    nc.vector.tensor_copy(dst, pz[:, :])

            def emit_store(g, tz, si):
                if not skip_dma:
                    eng = getattr(nc, store_eng)
                    sstep = cg * 2 * W // hstore
                    eng.dma_start(
                        out=out[g][:, si * sstep:(si + 1) * sstep],
                        in_=tz[:, si * sstep:(si + 1) * sstep])

            def emit_all():
                # flat software pipeline across group boundaries: pass1 of
                # pair k+1 is emitted before pass2 of pair k so the
                # in-order PE queue overlaps the DVE/ACT evacuations.
                spairs = npairs // hstore
                pend = []

                def drain():
                    g, j, ty, tz = pend.pop(0)
                    evac2(j, pass2(j, ty), tz)
                    if (j + 1) % spairs == 0:
                        emit_store(g, tz, (j + 1) // spairs - 1)

                for g in range(ng):
                    tx, tz = emit_loads(g)
                    for j in range(npairs) if not skip_compute else []:
                        pend.append((g, j, evac1(j, pass1(tx, j)), tz))
                        if len(pend) > 1:
                            drain()
                    if skip_compute:
                        for si in range(hstore):
                            emit_store(g, tz, si)
                while pend:
                    drain()

            if reps > 1:
                with tc.For_i(0, reps, 1):
                    emit_all()
            else:
                emit_all()
    nc.compile()
    return nc


def _get_nc(n_ch: int):
    key = (n_ch, CG)
    if key not in _CACHE:
        _CACHE[key] = _build(n_ch)
    return _CACHE[key]


def _prep_inputs(x: np.ndarray, k2: np.ndarray, n_ch: int, cg: int = CG):
    """Returns per-core input maps and the per-channel dequant scales."""
    ng = n_ch // cg
    kh, kw = _factor_kernel(k2)
    th = _toeplitz(H, kh).astype(ml_dtypes.bfloat16).reshape(2, P, H)
    tw = _toeplitz(W, kw).astype(ml_dtypes.bfloat16).reshape(2, P, W)
    th = np.ascontiguousarray(th)
    tw = np.ascontiguousarray(tw)

    x = np.asarray(x)
    # per-channel int8 output scale, folded into the input scaling
    amax = np.maximum(x.max(axis=(2, 3)), -x.min(axis=(2, 3)))  # [N, C]
    so = np.maximum(ALPHA * amax, 1e-30).astype(np.float32)
    xs = (x / so[:, :, None, None]).astype(ml_dtypes.bfloat16)
    # [n, (g c), (hb p), w] -> [n, g, p, c, hb, w]
    xb = xs.reshape(N, ng, cg, 2, P, W).transpose(0, 1, 4, 2, 3, 5)
    xin = np.ascontiguousarray(xb).reshape(N, ng, P, cg * 2 * W)

    in_maps = []
    for i in range(NCORES):
        in_maps.append({"xin": xin[i], "th": th, "tw": tw})
    return in_maps, so


def _unpack_out(raw: np.ndarray, so_n: np.ndarray, n_ch: int,
                cg: int = CG) -> np.ndarray:
    """[ng, P, cg*2*W] int8 -> [n_ch, H, W] f32; h = s*128 + p."""
    ng = n_ch // cg
    r = raw.reshape(ng, P, cg, 2, W).transpose(0, 2, 3, 1, 4)
    r = np.ascontiguousarray(r).reshape(n_ch, H, W).astype(np.float32)
    return r * so_n[:, None, None]


def _run(x: np.ndarray, k2: np.ndarray, trace: bool = False):
    n_ch = C
    nc = _get_nc(n_ch)
    in_maps, so = _prep_inputs(x, k2, n_ch)
    r = run_bass_kernel_spmd(nc, in_maps, core_ids=list(range(NCORES)),
                             trace=trace)
    outs = [_unpack_out(r.results[i]["out"], so[i], n_ch)
            for i in range(NCORES)]
    return np.stack(outs, axis=0), r


def kernel(x: np.ndarray, kernel: np.ndarray) -> np.ndarray:
    out, _ = _run(x, kernel, trace=False)
    return out


# revision 23
# speedup vs baseline: 1.7271x; 1.2436x over previous
"""TRN2 Bass kernel for nn_Blur: upfirdn2d(pad=(2,1)) with a separable 4x4
binomial FIR, x shape (8, 256, 256, 256) f32, depthwise per (n, c) plane.

Strategy (v2)
-------------
Batch-parallel across the 8 NeuronCores (core i gets x[i]).

Separable FIR as banded-Toeplitz matmuls on the TensorEngine, data
stationary in both passes (transpose-free):

  pass1:  Y^T = X^T @ T_H      pass2:  Z = Y @ T_W

v2 changes over the 223 us baseline (HW ~177 us, TimelineSim ~179 us;
floors: DMA 141 us for 48 MiB/core @ 358 GB/s, DVE+ACT PSUM
evacuations ~143 us):
- Band-pruned matmuls: each accumulation covers only the nonzero column
  ranges [0,130) (start=True) and [127,256) (start=False; has_written
  bits make the disjoint part overwrite and the 3-col overlap
  accumulate).  518 streamed cols per pass per channel instead of 770.
- int8 output: the per-channel quantization scale so_c = ALPHA*max|x_c|
  is folded into the host-side input scaling (x/so_c shipped as bf16),
  so the device just converts PSUM f32 -> int8 (hardware convert is
  round-to-nearest-even with saturation, probed).  Host dequantizes.
  Cuts store traffic 2x: 48 MiB/core total @ ~358 GB/s -> ~140 us.
- PSUM->SBUF evacuations batched to FD=1024 (two channels per
  instruction, PSUM tiles span 2 banks) and alternated between the DVE
  and ACT engines so neither is the bottleneck.
- cg=32 channels per DMA group (4 MiB loads / 2 MiB stores); loads on
  the SP HWDGE queue, stores on the Pool SWDGE queue so no compute
  engine's sequencer pays DMA-descriptor generation time.
"""
import numpy as np
import ml_dtypes

import concourse.bacc as bacc
import concourse.mybir as mybir
from concourse.tile import TileContext
from concourse.bass_utils import run_bass_kernel_spmd

N, C, H, W = 8, 256, 256, 256
P = 128          # partition size
NCORES = 8
# band: T[i, i+d] = k1[d+1], d in {-1, 0, 1, 2}
BAND_LO, BAND_HI = -1, 2
# nonzero column ranges of the two 128-row Toeplitz blocks
BLK_COLS = [(0, P + BAND_HI), (P + BAND_LO, 2 * P)]   # [0,130), [127,256)

CG = 32          # channels per DMA group
ALPHA = 0.010    # int8 output scale: so_c = ALPHA * max|x_c|

_CACHE = {}


def _factor_kernel(k2: np.ndarray):
    """Rank-1 factorization k2 = kh (x) kw (float64)."""
    k2 = np.asarray(k2, dtype=np.float64)
    u, s, vt = np.linalg.svd(k2)
    kh = u[:, 0] * np.sqrt(s[0])
    kw = vt[0] * np.sqrt(s[0])
    if kh.sum() < 0:
        kh, kw = -kh, -kw
    return kh, kw


def _toeplitz(n: int, k1: np.ndarray) -> np.ndarray:
    """T[i, j] = k1[j - i + 1] for 0 <= j-i+1 < 4, zero elsewhere."""
    t = np.zeros((n, n), dtype=np.float64)
    for d in range(BAND_LO, BAND_HI + 1):
        i = np.arange(max(0, -d), min(n, n - d))
        t[i, i + d] = k1[d + 1]
    return t


def _build(n_ch: int, cg: int = CG, reps: int = 1, *,
           bufs: tuple = (3, 4, 3), store_eng: str = "gpsimd",
           load_eng: str = "sync", hload: int = 4, hstore: int = 2,
           dve_per16: int = 7, evac_mode: str = "alt",
           act_extra: tuple = (7,), pybufs: int = 2, pzbufs: int = 2,
           pend_depth: int = 1, z_gran: int = 1024,
           skip_compute: bool = False, skip_dma: bool = False):
    """Build + compile the per-core Bass program (SPMD, one core's slice).

    reps > 1 repeats the whole channel loop (idempotent) in a hardware
    loop - a timing aid that amortizes dispatch overhead.
    """
    nc = bacc.Bacc("TRN2", target_bir_lowering=False)

    bf16 = mybir.dt.bfloat16
    f32 = mybir.dt.float32
    i8 = mybir.dt.int8

    assert n_ch % cg == 0
    ng = n_ch // cg
    # [group][partition][c][hb][w] pre-swizzled, pre-scaled bf16 input
    xin = nc.declare_dram_parameter("xin", [ng, P, cg * 2 * W], bf16,
                                    isOutput=False)
    th = nc.declare_dram_parameter("th", [2, P, H], bf16, isOutput=False)
    tw = nc.declare_dram_parameter("tw", [2, P, W], bf16, isOutput=False)
    # [group][partition][c][s][w] int8 output: h = s*128 + p, value
    # out[c,h,w]/so_c rounded (RNE, saturating)
    out = nc.declare_dram_parameter("out", [ng, P, cg * 2 * W], i8,
                                    isOutput=True)

    (lo0, hi0), (lo1, hi1) = BLK_COLS

    with TileContext(nc) as tc:
        with (tc.tile_pool(name="const", bufs=1) as cpool,
              tc.tile_pool(name="xin_p", bufs=bufs[0]) as xpool,
              tc.tile_pool(name="mid", bufs=bufs[1]) as mpool,
              tc.tile_pool(name="zout", bufs=bufs[2]) as zpool,
              tc.tile_pool(name="psy_a", bufs=1 if evac_mode == "streams"
                           else pybufs, space="PSUM") as pypool_a,
              tc.tile_pool(name="psz_a", bufs=1 if evac_mode == "streams"
                           else pzbufs, space="PSUM") as pzpool_a,
              tc.tile_pool(name="psy_b", bufs=1, space="PSUM") as pypool_b,
              tc.tile_pool(name="psz_b", bufs=1, space="PSUM") as pzpool_b):

            tth = [cpool.tile([P, H], bf16, name=f"tth{b}", tag=f"tth{b}")
                   for b in range(2)]
            ttw = [cpool.tile([P, W], bf16, name=f"ttw{b}", tag=f"ttw{b}")
                   for b in range(2)]
            for b in range(2):
                nc.sync.dma_start(out=tth[b][:, :], in_=th[b])
                nc.sync.dma_start(out=ttw[b][:, :], in_=tw[b])

            npairs = cg // 2

            def emit_loads(g):
                # load [128 x 32 KiB], sliced so compute starts after the
                # first slice lands instead of after the full 4 MiB
                tx = xpool.tile([P, cg * 2 * W], bf16, name="tx", tag="tx")
                if not skip_dma:
                    leng = getattr(nc, load_eng)
                    lstep = cg * 2 * W // hload
                    for si in range(hload):
                        leng.dma_start(
                            out=tx[:, si * lstep:(si + 1) * lstep],
                            in_=xin[g][:, si * lstep:(si + 1) * lstep])
                tz = zpool.tile([P, cg * 2 * W], i8, name="tz", tag="tz")
                if skip_compute:
                    nc.vector.memset(tz[:, :], 0)
                return tx, tz

            # Two independent evacuation streams: stream A's copies all run
            # on the DVE, stream B's on ACT.  Each stream owns 1-buf PSUM
            # tiles, so each stream's throughput is engine-bound rather
            # than (chain latency)/(buffer depth)-bound, and the two
            # engines never wait on each other.
            dve_slots = {i * 16 // dve_per16 for i in range(dve_per16)}

            def stream_of(j):
                return "a" if (j % 16) in dve_slots else "b"

            def e1_dve(j):
                # which engine runs evac1 of pair j (evac2 gets the other,
                # except act_extra pairs whose evac2 also goes to ACT)
                if evac_mode == "streams":
                    return stream_of(j) == "a"
                if evac_mode == "stage":      # evac1 on DVE, evac2 on ACT
                    return (j % 16) not in act_extra
                if evac_mode == "stage_r":
                    return (j % 16) in act_extra
                return j % 2 == 0

            def pass1(tx, j):
                # two channels (2j, 2j+1) -> py[:, q*512 + wb*256 + h']
                pool = pypool_b if (evac_mode == "streams"
                                    and stream_of(j) == "b") else pypool_a
                py = pool.tile([P, 1024], f32, name="py", tag="py")
                for q in range(2):
                    ci = 2 * j + q
                    for wb in range(2):
                        dst = q * 512 + wb * H
                        off = ci * 2 * W + wb * P
                        nc.tensor.matmul(
                            py[:, dst + lo0:dst + hi0],
                            tx[:, off:off + P],
                            tth[0][:, lo0:hi0],
                            start=True, stop=False)
                        nc.tensor.matmul(
                            py[:, dst + lo1:dst + hi1],
                            tx[:, off + W:off + W + P],
                            tth[1][:, lo1:hi1],
                            start=False, stop=True)
                return py

            def evac1(j, py):
                ty = mpool.tile([P, 1024], bf16, name="ty", tag="ty")
                if e1_dve(j):
                    nc.vector.tensor_copy(ty[:, :], py[:, :])
                else:
                    nc.scalar.copy(ty[:, :], py[:, :])
                return ty

            def pass2(j, ty):
                # pz[:, q*512 + s*256 + w'] ; partitions = h' in s
                pool = pzpool_b if (evac_mode == "streams"
                                    and stream_of(j) == "b") else pzpool_a
                nq = z_gran // 512
                pzs = []
                for q0 in range(0, 2, nq):
                    pz = pool.tile([P, z_gran], f32, name="pz", tag="pz")
                    pzs.append(pz)
                    for dq in range(nq):
                        q = q0 + dq
                        for s in range(2):
                            dst = dq * 512 + s * W
                            nc.tensor.matmul(
                                pz[:, dst + lo0:dst + hi0],
                                ty[:, q * 512 + 0 * H + s * P:
                                   q * 512 + 0 * H + s * P + P],
                                ttw[0][:, lo0:hi0],
                                start=True, stop=False)
                            nc.tensor.matmul(
                                pz[:, dst + lo1:dst + hi1],
                                ty[:, q * 512 + 1 * H + s * P:
                                   q * 512 + 1 * H + s * P + P],
                                ttw[1][:, lo1:hi1],
                                start=False, stop=True)
                return pzs

            def evac2(j, pzs, tz):
                if evac_mode == "streams":
                    dve2 = stream_of(j) == "a"
                elif evac_mode == "stage":
                    dve2 = False
                elif evac_mode == "stage_r":
                    dve2 = True
                else:
                    dve2 = (j % 2 == 1) and (j % 16) not in act_extra
                for i, pz in enumerate(pzs):
                    dst = tz[:, (2 * j) * 512 + i * z_gran:
                             (2 * j) * 512 + (i + 1) * z_gran]
                    if dve2:
                        nc.vector.tensor_copy(dst, pz[:, :])
                    else:
                        nc.scalar.copy(dst, pz[:, :])

            def emit_store(g, tz, si):
                if not skip_dma:
                    eng = getattr(nc, store_eng)
                    sstep = cg * 2 * W // hstore
                    eng.dma_start(
                        out=out[g][:, si * sstep:(si + 1) * sstep],
                        in_=tz[:, si * sstep:(si + 1) * sstep])

            def emit_all():
                # flat software pipeline across group boundaries: pass1 of
                # pair k+1 is emitted before pass2 of pair k so the
                # in-order PE queue overlaps the DVE/ACT evacuations.
                spairs = npairs // hstore
                pend = []

                def drain():
                    g, j, ty, tz = pend.pop(0)
                    evac2(j, pass2(j, ty), tz)
                    if (j + 1) % spairs == 0:
                        emit_store(g, tz, (j + 1) // spairs - 1)

                for g in range(ng):
                    tx, tz = emit_loads(g)
                    for j in range(npairs) if not skip_compute else []:
                        pend.append((g, j, evac1(j, pass1(tx, j)), tz))
                        if len(pend) > pend_depth:
                            drain()
                    if skip_compute:
                        for si in range(hstore):
                            emit_store(g, tz, si)
                while pend:
                    drain()

            if reps > 1:
                with tc.For_i(0, reps, 1):
                    emit_all()
            else:
                emit_all()
    nc.compile()
    return nc


def _get_nc(n_ch: int):
    key = (n_ch, CG)
    if key not in _CACHE:
        _CACHE[key] = _build(n_ch)
    return _CACHE[key]


def _prep_inputs(x: np.ndarray, k2: np.ndarray, n_ch: int, cg: int = CG):
    """Returns per-core input maps and the per-channel dequant scales."""
    ng = n_ch // cg
    kh, kw = _factor_kernel(k2)
    th = _toeplitz(H, kh).astype(ml_dtypes.bfloat16).reshape(2, P, H)
    tw = _toeplitz(W, kw).astype(ml_dtypes.bfloat16).reshape(2, P, W)
    th = np.ascontiguousarray(th)
    tw = np.ascontiguousarray(tw)

    x = np.asarray(x)
    # per-channel int8 output scale, folded into the input scaling
    amax = np.maximum(x.max(axis=(2, 3)), -x.min(axis=(2, 3)))  # [N, C]
    so = np.maximum(ALPHA * amax, 1e-30).astype(np.float32)
    xs = (x / so[:, :, None, None]).astype(ml_dtypes.bfloat16)
    # [n, (g c), (hb p), w] -> [n, g, p, c, hb, w]
    xb = xs.reshape(N, ng, cg, 2, P, W).transpose(0, 1, 4, 2, 3, 5)
    xin = np.ascontiguousarray(xb).reshape(N, ng, P, cg * 2 * W)

    in_maps = []
    for i in range(NCORES):
        in_maps.append({"xin": xin[i], "th": th, "tw": tw})
    return in_maps, so


def _unpack_out(raw: np.ndarray, so_n: np.ndarray, n_ch: int,
                cg: int = CG) -> np.ndarray:
    """[ng, P, cg*2*W] int8 -> [n_ch, H, W] f32; h = s*128 + p."""
    ng = n_ch // cg
    r = raw.reshape(ng, P, cg, 2, W).transpose(0, 2, 3, 1, 4)
    r = np.ascontiguousarray(r).reshape(n_ch, H, W).astype(np.float32)
    return r * so_n[:, None, None]


def _run(x: np.ndarray, k2: np.ndarray, trace: bool = False):
    n_ch = C
    nc = _get_nc(n_ch)
    in_maps, so = _prep_inputs(x, k2, n_ch)
    r = run_bass_kernel_spmd(nc, in_maps, core_ids=list(range(NCORES)),
                             trace=trace)
    outs = [_unpack_out(r.results[i]["out"], so[i], n_ch)
            for i in range(NCORES)]
    return np.stack(outs, axis=0), r


def kernel(x: np.ndarray, kernel: np.ndarray) -> np.ndarray:
    out, _ = _run(x, kernel, trace=False)
    return out


# revision 25
# speedup vs baseline: 1.7296x; 1.0014x over previous
"""TRN2 Bass kernel for nn_Blur: upfirdn2d(pad=(2,1)) with a separable 4x4
binomial FIR, x shape (8, 256, 256, 256) f32, depthwise per (n, c) plane.

Strategy (v2)
-------------
Batch-parallel across the 8 NeuronCores (core i gets x[i]).

Separable FIR as banded-Toeplitz matmuls on the TensorEngine, data
stationary in both passes (transpose-free):

  pass1:  Y^T = X^T @ T_H      pass2:  Z = Y @ T_W

v2 changes over the 223 us baseline (HW ~177 us, TimelineSim ~179 us;
floors: DMA 141 us for 48 MiB/core @ 358 GB/s, DVE+ACT PSUM
evacuations ~143 us):
- Band-pruned matmuls: each accumulation covers only the nonzero column
  ranges [0,130) (start=True) and [127,256) (start=False; has_written
  bits make the disjoint part overwrite and the 3-col overlap
  accumulate).  518 streamed cols per pass per channel instead of 770.
- int8 output: the per-channel quantization scale so_c = ALPHA*max|x_c|
  is folded into the host-side input scaling (x/so_c shipped as bf16),
  so the device just converts PSUM f32 -> int8 (hardware convert is
  round-to-nearest-even with saturation, probed).  Host dequantizes.
  Cuts store traffic 2x: 48 MiB/core total @ ~358 GB/s -> ~140 us.
- PSUM->SBUF evacuations batched to FD=1024 (two channels per
  instruction, PSUM tiles span 2 banks) and alternated between the DVE
  and ACT engines so neither is the bottleneck.
- cg=32 channels per DMA group (4 MiB loads / 2 MiB stores); loads on
  the SP HWDGE queue, stores on the Pool SWDGE queue so no compute
  engine's sequencer pays DMA-descriptor generation time.
"""
import numpy as np
import ml_dtypes

import concourse.bacc as bacc
import concourse.mybir as mybir
from concourse.tile import TileContext
from concourse.bass_utils import run_bass_kernel_spmd

N, C, H, W = 8, 256, 256, 256
P = 128          # partition size
NCORES = 8
# band: T[i, i+d] = k1[d+1], d in {-1, 0, 1, 2}
BAND_LO, BAND_HI = -1, 2
# nonzero column ranges of the two 128-row Toeplitz blocks
BLK_COLS = [(0, P + BAND_HI), (P + BAND_LO, 2 * P)]   # [0,130), [127,256)

CG = 32          # channels per DMA group
ALPHA = 0.010    # int8 output scale: so_c = ALPHA * max|x_c|

_CACHE = {}


def _factor_kernel(k2: np.ndarray):
    """Rank-1 factorization k2 = kh (x) kw (float64)."""
    k2 = np.asarray(k2, dtype=np.float64)
    u, s, vt = np.linalg.svd(k2)
    kh = u[:, 0] * np.sqrt(s[0])
    kw = vt[0] * np.sqrt(s[0])
    if kh.sum() < 0:
        kh, kw = -kh, -kw
    return kh, kw


def _toeplitz(n: int, k1: np.ndarray) -> np.ndarray:
    """T[i, j] = k1[j - i + 1] for 0 <= j-i+1 < 4, zero elsewhere."""
    t = np.zeros((n, n), dtype=np.float64)
    for d in range(BAND_LO, BAND_HI + 1):
        i = np.arange(max(0, -d), min(n, n - d))
        t[i, i + d] = k1[d + 1]
    return t


def _build(n_ch: int, cg: int = CG, reps: int = 1, *,
           bufs: tuple = (3, 4, 3), store_eng: str = "gpsimd",
           load_eng: str = "sync", hload: int = 4, hstore: int = 2,
           dve_per16: int = 7, evac_mode: str = "alt",
           act_extra: tuple = (7,), pybufs: int = 2, pzbufs: int = 2,
           pend_depth: int = 1, z_gran: int = 1024,
           skip_compute: bool = False, skip_dma: bool = False):
    """Build + compile the per-core Bass program (SPMD, one core's slice).

    reps > 1 repeats the whole channel loop (idempotent) in a hardware
    loop - a timing aid that amortizes dispatch overhead.
    """
    nc = bacc.Bacc("TRN2", target_bir_lowering=False)

    bf16 = mybir.dt.bfloat16
    f32 = mybir.dt.float32
    i8 = mybir.dt.int8

    assert n_ch % cg == 0
    ng = n_ch // cg
    # [group][partition][c][hb][w] pre-swizzled, pre-scaled bf16 input
    xin = nc.declare_dram_parameter("xin", [ng, P, cg * 2 * W], bf16,
                                    isOutput=False)
    th = nc.declare_dram_parameter("th", [2, P, H], bf16, isOutput=False)
    tw = nc.declare_dram_parameter("tw", [2, P, W], bf16, isOutput=False)
    # [group][partition][c][s][w] int8 output: h = s*128 + p, value
    # out[c,h,w]/so_c rounded (RNE, saturating)
    out = nc.declare_dram_parameter("out", [ng, P, cg * 2 * W], i8,
                                    isOutput=True)

    (lo0, hi0), (lo1, hi1) = BLK_COLS

    with TileContext(nc) as tc:
        with (tc.tile_pool(name="const", bufs=1) as cpool,
              tc.tile_pool(name="xin_p", bufs=bufs[0]) as xpool,
              tc.tile_pool(name="mid", bufs=bufs[1]) as mpool,
              tc.tile_pool(name="zout", bufs=bufs[2]) as zpool,
              tc.tile_pool(name="psy_a", bufs=1 if evac_mode == "streams"
                           else pybufs, space="PSUM") as pypool_a,
              tc.tile_pool(name="psz_a", bufs=1 if evac_mode == "streams"
                           else pzbufs, space="PSUM") as pzpool_a,
              tc.tile_pool(name="psy_b", bufs=1, space="PSUM") as pypool_b,
              tc.tile_pool(name="psz_b", bufs=1, space="PSUM") as pzpool_b):

            tth = [cpool.tile([P, H], bf16, name=f"tth{b}", tag=f"tth{b}")
                   for b in range(2)]
            ttw = [cpool.tile([P, W], bf16, name=f"ttw{b}", tag=f"ttw{b}")
                   for b in range(2)]
            for b in range(2):
                nc.sync.dma_start(out=tth[b][:, :], in_=th[b])
                nc.sync.dma_start(out=ttw[b][:, :], in_=tw[b])

            npairs = cg // 2

            tx_shared = None
            if skip_dma:
                # compute-only ablation: one static input tile, memset
                # once outside the reps loop so every group reads it
                tx_shared = cpool.tile([P, cg * 2 * W], bf16,
                                       name="tx0", tag="tx0")
                nc.vector.memset(tx_shared[:, :], 0.0)

            def emit_loads(g):
                # load [128 x 32 KiB], sliced so compute starts after the
                # first slice lands instead of after the full 4 MiB
                if skip_dma:
                    tz = zpool.tile([P, cg * 2 * W], i8, name="tz", tag="tz")
                    return tx_shared, tz
                tx = xpool.tile([P, cg * 2 * W], bf16, name="tx", tag="tx")
                if True:
                    lstep = cg * 2 * W // hload
                    for si in range(hload):
                        if load_eng == "alt":   # both HWDGE rings
                            leng = nc.sync if si % 2 == 0 else nc.scalar
                        else:
                            leng = getattr(nc, load_eng)
                        leng.dma_start(
                            out=tx[:, si * lstep:(si + 1) * lstep],
                            in_=xin[g][:, si * lstep:(si + 1) * lstep])
                tz = zpool.tile([P, cg * 2 * W], i8, name="tz", tag="tz")
                if skip_compute:
                    nc.vector.memset(tz[:, :], 0)
                return tx, tz

            # Two independent evacuation streams: stream A's copies all run
            # on the DVE, stream B's on ACT.  Each stream owns 1-buf PSUM
            # tiles, so each stream's throughput is engine-bound rather
            # than (chain latency)/(buffer depth)-bound, and the two
            # engines never wait on each other.
            dve_slots = {i * 16 // dve_per16 for i in range(dve_per16)}

            def stream_of(j):
                return "a" if (j % 16) in dve_slots else "b"

            def e1_dve(j):
                # which engine runs evac1 of pair j (evac2 gets the other,
                # except act_extra pairs whose evac2 also goes to ACT)
                if evac_mode == "streams":
                    return stream_of(j) == "a"
                if evac_mode == "stage":      # evac1 on DVE, evac2 on ACT
                    return (j % 16) not in act_extra
                if evac_mode == "stage_r":
                    return (j % 16) in act_extra
                return j % 2 == 0

            def pass1(tx, j):
                # two channels (2j, 2j+1) -> py[:, q*512 + wb*256 + h']
                pool = pypool_b if (evac_mode == "streams"
                                    and stream_of(j) == "b") else pypool_a
                py = pool.tile([P, 1024], f32, name="py", tag="py")
                for q in range(2):
                    ci = 2 * j + q
                    for wb in range(2):
                        dst = q * 512 + wb * H
                        off = ci * 2 * W + wb * P
                        nc.tensor.matmul(
                            py[:, dst + lo0:dst + hi0],
                            tx[:, off:off + P],
                            tth[0][:, lo0:hi0],
                            start=True, stop=False)
                        nc.tensor.matmul(
                            py[:, dst + lo1:dst + hi1],
                            tx[:, off + W:off + W + P],
                            tth[1][:, lo1:hi1],
                            start=False, stop=True)
                return py

            def evac1(j, py):
                ty = mpool.tile([P, 1024], bf16, name="ty", tag="ty")
                if e1_dve(j):
                    nc.vector.tensor_copy(ty[:, :], py[:, :])
                else:
                    nc.scalar.copy(ty[:, :], py[:, :])
                return ty

            def pass2(j, ty):
                # pz[:, q*512 + s*256 + w'] ; partitions = h' in s
                pool = pzpool_b if (evac_mode == "streams"
                                    and stream_of(j) == "b") else pzpool_a
                nq = z_gran // 512
                pzs = []
                for q0 in range(0, 2, nq):
                    pz = pool.tile([P, z_gran], f32, name="pz", tag="pz")
                    pzs.append(pz)
                    for dq in range(nq):
                        q = q0 + dq
                        for s in range(2):
                            dst = dq * 512 + s * W
                            nc.tensor.matmul(
                                pz[:, dst + lo0:dst + hi0],
                                ty[:, q * 512 + 0 * H + s * P:
                                   q * 512 + 0 * H + s * P + P],
                                ttw[0][:, lo0:hi0],
                                start=True, stop=False)
                            nc.tensor.matmul(
                                pz[:, dst + lo1:dst + hi1],
                                ty[:, q * 512 + 1 * H + s * P:
                                   q * 512 + 1 * H + s * P + P],
                                ttw[1][:, lo1:hi1],
                                start=False, stop=True)
                return pzs

            def evac2(j, pzs, tz):
                if evac_mode == "streams":
                    dve2 = stream_of(j) == "a"
                elif evac_mode == "stage":
                    dve2 = False
                elif evac_mode == "stage_r":
                    dve2 = True
                else:
                    dve2 = (j % 2 == 1) and (j % 16) not in act_extra
                for i, pz in enumerate(pzs):
                    dst = tz[:, (2 * j) * 512 + i * z_gran:
                             (2 * j) * 512 + (i + 1) * z_gran]
                    if dve2:
                        nc.vector.tensor_copy(dst, pz[:, :])
                    else:
                        nc.scalar.copy(dst, pz[:, :])

            def emit_store(g, tz, si):
                if not skip_dma:
                    eng = getattr(nc, store_eng)
                    sstep = cg * 2 * W // hstore
                    eng.dma_start(
                        out=out[g][:, si * sstep:(si + 1) * sstep],
                        in_=tz[:, si * sstep:(si + 1) * sstep])

            def emit_all():
                # flat software pipeline across group boundaries: pass1 of
                # pair k+1 is emitted before pass2 of pair k so the
                # in-order PE queue overlaps the DVE/ACT evacuations.
                spairs = npairs // hstore
                pend = []

                def drain():
                    g, j, ty, tz = pend.pop(0)
                    evac2(j, pass2(j, ty), tz)
                    if (j + 1) % spairs == 0:
                        emit_store(g, tz, (j + 1) // spairs - 1)

                for g in range(ng):
                    tx, tz = emit_loads(g)
                    for j in range(npairs) if not skip_compute else []:
                        pend.append((g, j, evac1(j, pass1(tx, j)), tz))
                        if len(pend) > pend_depth:
                            drain()
                    if skip_compute:
                        for si in range(hstore):
                            emit_store(g, tz, si)
                while pend:
                    drain()

            if reps > 1:
                with tc.For_i(0, reps, 1):
                    emit_all()
            else:
                emit_all()
    nc.compile()
    return nc


def _get_nc(n_ch: int):
    key = (n_ch, CG)
    if key not in _CACHE:
        _CACHE[key] = _build(n_ch)
    return _CACHE[key]


def _prep_inputs(x: np.ndarray, k2: np.ndarray, n_ch: int, cg: int = CG):
    """Returns per-core input maps and the per-channel dequant scales."""
    ng = n_ch // cg
    kh, kw = _factor_kernel(k2)
    th = _toeplitz(H, kh).astype(ml_dtypes.bfloat16).reshape(2, P, H)
    tw = _toeplitz(W, kw).astype(ml_dtypes.bfloat16).reshape(2, P, W)
    th = np.ascontiguousarray(th)
    tw = np.ascontiguousarray(tw)

    x = np.asarray(x)
    # per-channel int8 output scale, folded into the input scaling
    amax = np.maximum(x.max(axis=(2, 3)), -x.min(axis=(2, 3)))  # [N, C]
    so = np.maximum(ALPHA * amax, 1e-30).astype(np.float32)
    xs = (x / so[:, :, None, None]).astype(ml_dtypes.bfloat16)
    # [n, (g c), (hb p), w] -> [n, g, p, c, hb, w]
    xb = xs.reshape(N, ng, cg, 2, P, W).transpose(0, 1, 4, 2, 3, 5)
    xin = np.ascontiguousarray(xb).reshape(N, ng, P, cg * 2 * W)

    in_maps = []
    for i in range(NCORES):
        in_maps.append({"xin": xin[i], "th": th, "tw": tw})
    return in_maps, so


def _unpack_out(raw: np.ndarray, so_n: np.ndarray, n_ch: int,
                cg: int = CG) -> np.ndarray:
    """[ng, P, cg*2*W] int8 -> [n_ch, H, W] f32; h = s*128 + p."""
    ng = n_ch // cg
    r = raw.reshape(ng, P, cg, 2, W).transpose(0, 2, 3, 1, 4)
    r = np.ascontiguousarray(r).reshape(n_ch, H, W).astype(np.float32)
    return r * so_n[:, None, None]


def _run(x: np.ndarray, k2: np.ndarray, trace: bool = False):
    n_ch = C
    nc = _get_nc(n_ch)
    in_maps, so = _prep_inputs(x, k2, n_ch)
    r = run_bass_kernel_spmd(nc, in_maps, core_ids=list(range(NCORES)),
                             trace=trace)
    outs = [_unpack_out(r.results[i]["out"], so[i], n_ch)
            for i in range(NCORES)]
    return np.stack(outs, axis=0), r


def kernel(x: np.ndarray, kernel: np.ndarray) -> np.ndarray:
    out, _ = _run(x, kernel, trace=False)
    return out
